# revision 1
# baseline (speedup 1.0000x reference)
"""Bass/Trainium2 kernel for the BilinearInteractionLayer problem.

out[b, p, f] = (sum_e emb[b, I[p], e] * W[p, f, e]) * emb[b, J[p], f]
  emb: [2048, 40, 64] f32, W: [780, 64, 64] f32, out: [2048, 780, 64] f32

Strategy (data parallel over batch, 8 cores x 256 rows):
  - Pairs (i, j) grouped by i ("blocks"; block i has 39-i pairs, consecutive p).
    Blocks split into two 390-pair halves (A: i in 0..9 + 30..38, B: i in
    10..29) assigned to PE row-groups 0-63 / 64-127 so two K=64 matmuls run
    concurrently on the 128x128 array.
  - Per half, a "tape" of 390*64 = 24960 (pair, f) columns; W is pre-arranged
    on host to [128, 24960] (partition = e for half A rows 0-63 / half B rows
    64-127) and streamed in 13 chunks of 1920 cols (~0.98 MB DMAs).
  - matmul: lhsT = embT[e, b] (stationary, [64, 128] per batch-chunk),
    rhs = W chunk slice [64, <=512], out psum[b, (pair, f)].
  - VectorE multiplies psum by emb[b, j, f] (contiguous slice of the natural
    layout) writing SBUF out tiles, DMA'd to HBM in tape order.
  - Host reorders tape pair order -> global pair order at the end.
"""

import os
import numpy as np

import concourse.bass as bass
import concourse.mybir as mybir
import concourse.tile as tile
from concourse import bacc
from concourse import bass_utils

F32 = mybir.dt.float32

NUM_FIELDS = 40
EMBED = 64
BATCH = 2048
NCORES = 8
BL = BATCH // NCORES          # 256 rows per core
BCHUNKS = 2                   # 2 x 128 partition chunks of the local batch
NPAIRS = 780

HALVES = [list(range(0, 10)) + list(range(30, 39)), list(range(10, 30))]
HALF_PAIRS = 390
TAPE = HALF_PAIRS * EMBED     # 24960 cols per half
CHUNK = 1920                  # W/out tile width (cols); 13 chunks per tape
NCHUNK = TAPE // CHUNK
PSGRID = 1024                 # psum tile width (2 banks)
MMMAX = 512                   # max matmul free dim (one psum bank, fp32)

assert TAPE % CHUNK == 0


def _half_blocks(h):
    """[(i, tape_start_col, ncols)] for half h, in tape order."""
    res = []
    pos = 0
    for i in HALVES[h]:
        cols = (NUM_FIELDS - 1 - i) * EMBED
        res.append((i, pos, cols))
        pos += cols
    assert pos == TAPE
    return res


def _chunk_groups(h, c):
    """Groups for chunk c of half h: (i, abs_start, cols, j0).

    Split at block boundaries and at the PSGRID grid (relative to the chunk
    start) so each group fits one psum tile; j0 is the first j of the group.
    """
    c0, c1 = c * CHUNK, (c + 1) * CHUNK
    groups = []
    for (i, b0, bcols) in _half_blocks(h):
        lo, hi = max(b0, c0), min(b0 + bcols, c1)
        s = lo
        while s < hi:
            nxt = c0 + ((s - c0) // PSGRID + 1) * PSGRID
            e = min(hi, nxt)
            j0 = i + 1 + (s - b0) // EMBED
            groups.append((i, s, e - s, j0))
            s = e
    return groups


def _pairs_tape():
    """Global pair indices (combinations order) in tape order: half A then B."""
    pidx = {}
    k = 0
    for i in range(NUM_FIELDS):
        for j in range(i + 1, NUM_FIELDS):
            pidx[(i, j)] = k
            k += 1
    order = []
    for h in (0, 1):
        for i in HALVES[h]:
            for j in range(i + 1, NUM_FIELDS):
                order.append(pidx[(i, j)])
    return np.array(order, dtype=np.int64)


def _build_nc():
    nc = bacc.Bacc("TRN2", target_bir_lowering=False, debug=False)

    wt_d = nc.dram_tensor("Wt", [128, TAPE], F32, kind="ExternalInput")
    embT_d = nc.dram_tensor("embT", [128, NUM_FIELDS * BL], F32, kind="ExternalInput")
    embN_d = nc.dram_tensor("embN", [128, BCHUNKS * NUM_FIELDS * EMBED], F32,
                            kind="ExternalInput")
    out_d = nc.dram_tensor("out", [BL, 2 * TAPE], F32, kind="ExternalOutput")

    wt_ap, embT_ap, embN_ap, out_ap = (
        wt_d.ap(), embT_d.ap(), embN_d.ap(), out_d.ap())

    NF = NUM_FIELDS * EMBED  # 2560, embN cols per batch chunk

    with tile.TileContext(nc) as tc:
        with (
            tc.tile_pool(name="const", bufs=1) as cpool,
            tc.tile_pool(name="w", bufs=3) as wpool,
            tc.tile_pool(name="o", bufs=6) as opool,
            tc.tile_pool(name="ps", bufs=4, space="PSUM") as ppool,
        ):
            embT_s = cpool.tile([128, NUM_FIELDS * BL], F32)
            nc.sync.dma_start(embT_s[:], embT_ap[:])
            embN_s = cpool.tile([128, BCHUNKS * NF], F32)
            nc.sync.dma_start(embN_s[:], embN_ap[:])

            for c in range(NCHUNK):
                wt = wpool.tile([128, CHUNK], F32, tag="w")
                nc.sync.dma_start(wt[:], wt_ap[:, c * CHUNK:(c + 1) * CHUNK])
                groups_h = [_chunk_groups(0, c), _chunk_groups(1, c)]
                for bc in range(BCHUNKS):
                    otiles = [opool.tile([128, CHUNK], F32, tag="o", name=f"o{c}_{bc}_{h}")
                              for h in range(2)]

                    def emit_half(h):
                        for (i, gs, gcols, j0) in groups_h[h]:
                            pt = ppool.tile([128, PSGRID], F32, tag="ps",
                                            name=f"ps{c}_{bc}_{h}_{gs}")
                            s = 0
                            while s < gcols:
                                w = min(MMMAX, gcols - s)
                                yield ("mm", (h, pt, i, gs, s, w))
                                s += w
                            yield ("mul", (h, pt, gs, gcols, j0))

                    streams = [emit_half(0), emit_half(1)]
                    done = [False, False]
                    turn = 0
                    while not all(done):
                        if done[turn]:
                            turn ^= 1
                        try:
                            kind, args = next(streams[turn])
                        except StopIteration:
                            done[turn] = True
                            turn ^= 1
                            continue
                        if kind == "mm":
                            h, pt, i, gs, s, w = args
                            rel = gs - c * CHUNK
                            col0 = i * BL + bc * 128
                            nc.tensor.matmul(
                                pt[:, s:s + w],
                                lhsT=embT_s[h * 64:(h + 1) * 64, col0:col0 + 128],
                                rhs=wt[h * 64:(h + 1) * 64, rel + s:rel + s + w],
                                start=True, stop=True,
                            )
                            # alternate halves between matmuls for row-group
                            # concurrency on the PE array
                            turn ^= 1
                        else:
                            h, pt, gs, gcols, j0 = args
                            rel = gs - c * CHUNK
                            nc.vector.tensor_mul(
                                otiles[h][:, rel:rel + gcols],
                                pt[:, :gcols],
                                embN_s[:, bc * NF + j0 * EMBED:
                                       bc * NF + j0 * EMBED + gcols],
                            )
                    for h in range(2):
                        nc.scalar.dma_start(
                            out_ap[bc * 128:(bc + 1) * 128,
                                   h * TAPE + c * CHUNK:
                                   h * TAPE + (c + 1) * CHUNK],
                            otiles[h][:],
                        )

    nc.compile()
    return nc


_NC = None
_TAPE_ORDER = None
LAST_RESULT = None


def kernel(feature_emb, W):
    global _NC, _TAPE_ORDER, LAST_RESULT
    feature_emb = np.ascontiguousarray(feature_emb, dtype=np.float32)
    W = np.ascontiguousarray(W, dtype=np.float32)
    assert feature_emb.shape == (BATCH, NUM_FIELDS, EMBED)
    assert W.shape == (NPAIRS, EMBED, EMBED)

    if _NC is None:
        _NC = _build_nc()
        _TAPE_ORDER = _pairs_tape()

    # W tape: [128, 24960]; rows 0-63 half A (partition = e), rows 64-127 half B
    wsel = W[_TAPE_ORDER]                       # [780, 64(f), 64(e)] tape order
    wa = wsel[:HALF_PAIRS].transpose(2, 0, 1).reshape(EMBED, TAPE)
    wb = wsel[HALF_PAIRS:].transpose(2, 0, 1).reshape(EMBED, TAPE)
    wt = np.ascontiguousarray(np.concatenate([wa, wb], axis=0))

    in_maps = []
    for c in range(NCORES):
        ec = feature_emb[c * BL:(c + 1) * BL]   # [256, 40, 64]
        t1 = ec.transpose(2, 1, 0).reshape(EMBED, NUM_FIELDS * BL)  # [e,(i,b)]
        embT = np.ascontiguousarray(np.concatenate([t1, t1], axis=0))
        embN = np.ascontiguousarray(
            ec.reshape(BCHUNKS, 128, NUM_FIELDS * EMBED)
              .transpose(1, 0, 2).reshape(128, BCHUNKS * NUM_FIELDS * EMBED))
        in_maps.append({"Wt": wt, "embT": embT, "embN": embN})

    trace = bool(int(os.environ.get("BILIN_TRACE", "0")))
    res = bass_utils.run_bass_kernel_spmd(
        _NC, in_maps, core_ids=list(range(NCORES)), trace=trace)
    LAST_RESULT = res

    out = np.empty((BATCH, NPAIRS, EMBED), dtype=np.float32)
    for c in range(NCORES):
        t = np.asarray(res.results[c]["out"]).reshape(BL, NPAIRS, EMBED)
        out[c * BL:(c + 1) * BL][:, _TAPE_ORDER, :] = t
    return out


# revision 7
# speedup vs baseline: 1.0650x; 1.0650x over previous
"""Bass/Trainium2 kernel for the BilinearInteractionLayer problem.

out[b, p, f] = (sum_e emb[b, I[p], e] * W[p, f, e]) * emb[b, J[p], f]
  emb: [2048, 40, 64] f32, W: [780, 64, 64] f32, out: [2048, 780, 64] f32

Strategy (data parallel over batch, 8 cores x 256 rows):
  - Pairs (i, j) grouped by i ("blocks"; block i has 39-i pairs, consecutive p).
    Blocks split into two 390-pair halves (A: i in 0..9 + 30..38, B: i in
    10..29) assigned to PE row-groups 0-63 / 64-127 so two K=64 matmuls run
    concurrently on the 128x128 array.
  - Per half, a "tape" of 390*64 = 24960 (pair, f) columns; W is pre-arranged
    on host to [128, 24960] (partition = e for half A rows 0-63 / half B rows
    64-127) and streamed in 13 chunks of 1920 cols (~0.98 MB DMAs).
  - matmul: lhsT = embT[e, b] (stationary, [64, 128] per batch-chunk),
    rhs = W chunk slice [64, <=512], out psum[b, (pair, f)].
  - VectorE multiplies psum by emb[b, j, f] (contiguous slice of the natural
    layout) writing SBUF out tiles, DMA'd to HBM in tape order.
  - Host reorders tape pair order -> global pair order at the end.
"""

import os
import numpy as np

import concourse.bass as bass
import concourse.mybir as mybir
import concourse.tile as tile
from concourse import bacc
from concourse import bass_utils

F32 = mybir.dt.float32

NUM_FIELDS = 40
EMBED = 64
BATCH = 2048
NCORES = 8
BL = BATCH // NCORES          # 256 rows per core
BCHUNKS = 2                   # 2 x 128 partition chunks of the local batch
NPAIRS = 780

HALVES = [list(range(0, 10)) + list(range(30, 39)), list(range(10, 30))]
HALF_PAIRS = 390
TAPE = HALF_PAIRS * EMBED     # 24960 cols per half
CHUNK = 2048                  # W/out tile width (cols); 12 full + 1 tail
NCHUNK = (TAPE + CHUNK - 1) // CHUNK
PSGRID = 1024                 # psum tile width (2 banks)
MMMAX = 512                   # max matmul free dim (one psum bank, fp32)


def _chunk_cols(c):
    return min(CHUNK, TAPE - c * CHUNK)


def _half_blocks(h):
    """[(i, tape_start_col, ncols)] for half h, in tape order."""
    res = []
    pos = 0
    for i in HALVES[h]:
        cols = (NUM_FIELDS - 1 - i) * EMBED
        res.append((i, pos, cols))
        pos += cols
    assert pos == TAPE
    return res


def _chunk_groups(h, c):
    """Groups for chunk c of half h: (i, abs_start, cols, j0).

    Split at block boundaries and at the PSGRID grid (relative to the chunk
    start) so each group fits one psum tile; j0 is the first j of the group.
    """
    c0, c1 = c * CHUNK, c * CHUNK + _chunk_cols(c)
    groups = []
    for (i, b0, bcols) in _half_blocks(h):
        lo, hi = max(b0, c0), min(b0 + bcols, c1)
        s = lo
        while s < hi:
            nxt = c0 + ((s - c0) // PSGRID + 1) * PSGRID
            e = min(hi, nxt)
            j0 = i + 1 + (s - b0) // EMBED
            groups.append((i, s, e - s, j0))
            s = e
    return groups


def _pairs_tape():
    """Global pair indices (combinations order) in tape order: half A then B."""
    pidx = {}
    k = 0
    for i in range(NUM_FIELDS):
        for j in range(i + 1, NUM_FIELDS):
            pidx[(i, j)] = k
            k += 1
    order = []
    for h in (0, 1):
        for i in HALVES[h]:
            for j in range(i + 1, NUM_FIELDS):
                order.append(pidx[(i, j)])
    return np.array(order, dtype=np.int64)


def _build_nc():
    nc = bacc.Bacc("TRN2", target_bir_lowering=False, debug=False)

    wt_d = nc.dram_tensor("Wt", [128, TAPE], F32, kind="ExternalInput")
    embT_d = nc.dram_tensor("embT", [64, NUM_FIELDS * BL], F32, kind="ExternalInput")
    embN_d = nc.dram_tensor("embN", [128, BCHUNKS * NUM_FIELDS * EMBED], F32,
                            kind="ExternalInput")
    out_d = nc.dram_tensor("out", [BL, 2 * TAPE], F32, kind="ExternalOutput")

    wt_ap, embT_ap, embN_ap, out_ap = (
        wt_d.ap(), embT_d.ap(), embN_d.ap(), out_d.ap())

    NF = NUM_FIELDS * EMBED  # 2560, embN cols per batch chunk

    with tile.TileContext(nc) as tc:
        with (
            tc.tile_pool(name="const", bufs=1) as cpool,
            tc.tile_pool(name="w", bufs=3) as wpool,
            tc.tile_pool(name="o", bufs=6) as opool,
            tc.tile_pool(name="ps", bufs=4, space="PSUM") as ppool,
        ):
            embT_s = cpool.tile([128, NUM_FIELDS * BL], F32)
            nc.sync.dma_start(embT_s[0:64, :], embT_ap[:])
            # duplicate into partitions 64-127 on-chip (SBUF->SBUF, no HBM)
            nc.sync.dma_start(embT_s[64:128, :], embT_s[0:64, :])
            embN_s = cpool.tile([128, BCHUNKS * NF], F32)
            nc.sync.dma_start(embN_s[:], embN_ap[:])

            for c in range(NCHUNK):
                ccols = _chunk_cols(c)
                wt = wpool.tile([128, CHUNK], F32, tag="w")
                nc.sync.dma_start(wt[:, :ccols],
                                  wt_ap[:, c * CHUNK:c * CHUNK + ccols])
                groups_h = [_chunk_groups(0, c), _chunk_groups(1, c)]
                for bc in range(BCHUNKS):
                    otiles = [opool.tile([128, CHUNK], F32, tag="o", name=f"o{c}_{bc}_{h}")
                              for h in range(2)]

                    def emit_half(h):
                        for (i, gs, gcols, j0) in groups_h[h]:
                            pt = ppool.tile([128, PSGRID], F32, tag="ps",
                                            name=f"ps{c}_{bc}_{h}_{gs}")
                            s = 0
                            while s < gcols:
                                w = min(MMMAX, gcols - s)
                                yield ("mm", (h, pt, i, gs, s, w))
                                s += w
                            yield ("mul", (h, pt, gs, gcols, j0))

                    streams = [emit_half(0), emit_half(1)]
                    done = [False, False]
                    turn = 0
                    while not all(done):
                        if done[turn]:
                            turn ^= 1
                        try:
                            kind, args = next(streams[turn])
                        except StopIteration:
                            done[turn] = True
                            turn ^= 1
                            continue
                        if kind == "mm":
                            h, pt, i, gs, s, w = args
                            rel = gs - c * CHUNK
                            col0 = i * BL + bc * 128
                            nc.tensor.matmul(
                                pt[:, s:s + w],
                                lhsT=embT_s[h * 64:(h + 1) * 64, col0:col0 + 128],
                                rhs=wt[h * 64:(h + 1) * 64, rel + s:rel + s + w],
                                start=True, stop=True,
                            )
                            # alternate halves between matmuls for row-group
                            # concurrency on the PE array
                            turn ^= 1
                        else:
                            h, pt, gs, gcols, j0 = args
                            rel = gs - c * CHUNK
                            nc.vector.tensor_mul(
                                otiles[h][:, rel:rel + gcols],
                                pt[:, :gcols],
                                embN_s[:, bc * NF + j0 * EMBED:
                                       bc * NF + j0 * EMBED + gcols],
                            )
                    for h in range(2):
                        nc.scalar.dma_start(
                            out_ap[bc * 128:(bc + 1) * 128,
                                   h * TAPE + c * CHUNK:
                                   h * TAPE + c * CHUNK + ccols],
                            otiles[h][:, :ccols],
                        )

    nc.compile()
    return nc


_NC = None
_TAPE_ORDER = None
LAST_RESULT = None


def kernel(feature_emb, W):
    global _NC, _TAPE_ORDER, LAST_RESULT
    feature_emb = np.ascontiguousarray(feature_emb, dtype=np.float32)
    W = np.ascontiguousarray(W, dtype=np.float32)
    assert feature_emb.shape == (BATCH, NUM_FIELDS, EMBED)
    assert W.shape == (NPAIRS, EMBED, EMBED)

    if _NC is None:
        _NC = _build_nc()
        _TAPE_ORDER = _pairs_tape()

    # W tape: [128, 24960]; rows 0-63 half A (partition = e), rows 64-127 half B
    wsel = W[_TAPE_ORDER]                       # [780, 64(f), 64(e)] tape order
    wa = wsel[:HALF_PAIRS].transpose(2, 0, 1).reshape(EMBED, TAPE)
    wb = wsel[HALF_PAIRS:].transpose(2, 0, 1).reshape(EMBED, TAPE)
    wt = np.ascontiguousarray(np.concatenate([wa, wb], axis=0))

    in_maps = []
    for c in range(NCORES):
        ec = feature_emb[c * BL:(c + 1) * BL]   # [256, 40, 64]
        embT = np.ascontiguousarray(
            ec.transpose(2, 1, 0).reshape(EMBED, NUM_FIELDS * BL))  # [e,(i,b)]
        embN = np.ascontiguousarray(
            ec.reshape(BCHUNKS, 128, NUM_FIELDS * EMBED)
              .transpose(1, 0, 2).reshape(128, BCHUNKS * NUM_FIELDS * EMBED))
        in_maps.append({"Wt": wt, "embT": embT, "embN": embN})

    trace = bool(int(os.environ.get("BILIN_TRACE", "0")))
    res = bass_utils.run_bass_kernel_spmd(
        _NC, in_maps, core_ids=list(range(NCORES)), trace=trace)
    LAST_RESULT = res

    out = np.empty((BATCH, NPAIRS, EMBED), dtype=np.float32)
    for c in range(NCORES):
        t = np.asarray(res.results[c]["out"]).reshape(BL, NPAIRS, EMBED)
        out[c * BL:(c + 1) * BL][:, _TAPE_ORDER, :] = t
    return out
